# revision 42
# baseline (speedup 1.0000x reference)
"""Banded multi-head attention on 8 Trainium2 NeuronCores.

Problem: B=2, L=2048, D=1024, H=16 heads, d_k=64. The band mask is a 0/1
FLOAT tensor *added* to the scores (not -inf masked), so the softmax is
dense; exp(s + band) = exp(s) * e^band is handled by multiplying constant
e-or-1 parallelogram tiles over the band region.

Sharding: (batch x 4-head-groups) across the 8 cores. Host pre-transposes
activations/weights so every device matmul has its contraction dim on
partitions; the per-core partial output projections are summed on the host
(gather/unshard) together with the output bias.

All device matmuls run on bf16 operands with f32 PSUM accumulation; the
tolerance (2e-2) leaves ample margin and bf16 is the PE's 1-col/cycle
fast path (fp32r lowers to the slow HIGH-precision mode on this stack).
"""

import sys

sys.path.insert(0, "/opt/trn_rl_repo")

import numpy as np
import ml_dtypes
from contextlib import ExitStack

import concourse.bass as bass
import concourse.tile as tile
from concourse import bacc, mybir
from concourse.bass_utils import run_bass_kernel_spmd

dt = mybir.dt
AF = mybir.ActivationFunctionType

B, L, D, H, DK = 2, 2048, 1024, 16, 64
HPC = 4            # heads per core
HD = HPC * DK      # 256: head dims per core
NQC, QCW = 4, 512  # q chunks
NKB, KBW = 16, 128 # k blocks
NDC, DCW = 8, 128  # D chunks
SCALE = 1.0 / 8.0  # 1/sqrt(d_k)

_CACHE = {}


def _band_slots(half):
    """delta -> (slot, c0, c1) for 128x512 tiles at k-offset kb*128, q-offset
    qc*512, delta = kb*128 - qc*512. Band cols: f in [delta-half, delta+127+half]."""
    slots = {}
    d = -((half + 127) // 128) * 128
    while d <= half + 511:
        c0, c1 = max(0, d - half), min(512, d + 128 + half)
        if c0 < c1:
            slots[d] = (len(slots), c0, c1)
        d += 128
    return slots


def _build(masksize, stop_after=None):
    half = int(masksize) // 2
    slots = _band_slots(half)
    ns = max(len(slots), 1)

    nc = bacc.Bacc("TRN2", target_bir_lowering=False, debug=False)

    f32, bf16 = dt.float32, dt.bfloat16
    xq = nc.dram_tensor("xq", [D, L], bf16, kind="ExternalInput").ap()
    xk = nc.dram_tensor("xk", [D, L], bf16, kind="ExternalInput").ap()
    xv = nc.dram_tensor("xv", [D, L], bf16, kind="ExternalInput").ap()
    # weights pre-packed on host into SBUF layouts (see _prep_inmaps)
    wq = nc.dram_tensor("wq", [128, NDC * HD], bf16, kind="ExternalInput").ap()
    wk = nc.dram_tensor("wk", [128, NDC * HD], bf16, kind="ExternalInput").ap()
    wv = nc.dram_tensor("wv", [128, NDC * HD], bf16, kind="ExternalInput").ap()
    wo = nc.dram_tensor("wo", [128, 2 * D], bf16, kind="ExternalInput").ap()
    bq = nc.dram_tensor("bq", [128, 2], f32, kind="ExternalInput").ap()
    bk = nc.dram_tensor("bk", [128, 2], f32, kind="ExternalInput").ap()
    bvt = nc.dram_tensor("bvt", [128, HD], f32, kind="ExternalInput").ap()
    em = nc.dram_tensor("em", [128, ns * 512], bf16, kind="ExternalInput").ap()
    on1 = nc.dram_tensor("on1", [1, 64], bf16, kind="ExternalInput").ap()
    yt = nc.dram_tensor("yt", [D, L], bf16, kind="ExternalOutput").ap()

    with tile.TileContext(nc) as tc, ExitStack() as ctx:
        ctx.enter_context(
            nc.allow_low_precision(reason="bf16 matmul operands are intentional")
        )
        # ---- persistent SBUF ----
        wts = ctx.enter_context(tc.tile_pool(name="wts", bufs=1))
        big = ctx.enter_context(tc.tile_pool(name="big", bufs=1))

        wq_sb = wts.tile([128, NDC * HD], bf16, tag="wq", name="wq")
        wk_sb = wts.tile([128, NDC * HD], bf16, tag="wk", name="wk")
        wv_sb = wts.tile([128, NDC * HD], bf16, tag="wv", name="wv")
        wo_sb = wts.tile([128, 2 * D], bf16, tag="wo", name="wo")
        bq_sb = wts.tile([128, 2], f32, tag="bq", name="bq")
        bk_sb = wts.tile([128, 2], f32, tag="bk", name="bk")
        bvt_sb = wts.tile([128, HD], f32, tag="bvt", name="bvt")
        em_sb = wts.tile([128, ns * 512], bf16, tag="em", name="em")
        on1_sb = wts.tile([1, 64], bf16, tag="on1", name="on1")
        # DMA issue order is managed below: phase-B weights first, then the
        # xq/xk stream, then everything else in phase order

        # projection outputs (resident): q/k in T-layout, out paired by heads
        qt_sb = [big.tile([128, L], bf16, tag=f"qt{t}", name=f"qt{t}") for t in range(2)]
        kt_sb = [big.tile([128, L], bf16, tag=f"kt{t}", name=f"kt{t}") for t in range(2)]
        otp_sb = [big.tile([128, L], bf16, tag=f"ot{t}", name=f"ot{t}") for t in range(2)]
        # v (natural layout) + ones col per head: [128, HPC*66] per k-block
        vaug_sb = [big.tile([128, HPC * 66], bf16, tag=f"vaug{lb}", name=f"vaug{lb}") for lb in range(NKB)]
        for lb in range(NKB):
            nc.gpsimd.memset(vaug_sb[lb][:], 1.0)

        # ---- phases B/C: projections with resident x chunks ----
        with tc.tile_pool(name="xs", bufs=1) as xs:
            # halves so phase B's first matmul (needs wq cols 0:256 only)
            # starts as soon as the first 256KB lands
            for t_sb, t_in in ((wq_sb, wq), (wk_sb, wk)):
                half = NDC * HD // 2
                nc.sync.dma_start(t_sb[:, 0:half], t_in[:, 0:half])
                nc.sync.dma_start(t_sb[:, half:], t_in[:, half:])
            # xq/xk chunks first (phase B needs them all); xv streams in
            # during phase B compute and is only needed for phase C
            xq_t, xk_t, xv_t = [], [], []
            for c in range(NDC):
                for lst, nm, src in ((xq_t, "xq", xq), (xk_t, "xk", xk)):
                    t_ = xs.tile([128, L], bf16, tag=f"{nm}{c}", name=f"{nm}{c}")
                    nc.sync.dma_start(t_[:], src[c * DCW:(c + 1) * DCW, :])
                    lst.append(t_)
            for t_sb, t_in in ((bq_sb, bq), (bk_sb, bk), (wv_sb, wv),
                               (bvt_sb, bvt), (em_sb, em)):
                nc.sync.dma_start(t_sb[:], t_in[:])
            for c in range(NDC):
                t_ = xs.tile([128, L], bf16, tag=f"xv{c}", name=f"xv{c}")
                nc.sync.dma_start(t_[:], xv[c * DCW:(c + 1) * DCW, :])
                xv_t.append(t_)
            for t_sb, t_in in ((wo_sb, wo), (on1_sb, on1)):
                nc.sync.dma_start(t_sb[:], t_in[:])

            # phase B: q/k projections (T-layout)
            with tc.tile_pool(name="pbk", bufs=2, space="PSUM") as pbk:
                for qc in range(NQC):
                    pq = [pbk.tile([128, QCW], f32, tag=f"pq{t}", name=f"pq{t}") for t in range(2)]
                    pk = [pbk.tile([128, QCW], f32, tag=f"pk{t}", name=f"pk{t}") for t in range(2)]
                    for c in range(NDC):
                        for t in range(2):
                            nc.tensor.matmul(
                                pq[t][:], wq_sb[:, c * HD + t * 128: c * HD + (t + 1) * 128],
                                xq_t[c][:, qc * QCW:(qc + 1) * QCW],
                                start=(c == 0), stop=(c == NDC - 1),
                            )
                            nc.tensor.matmul(
                                pk[t][:], wk_sb[:, c * HD + t * 128: c * HD + (t + 1) * 128],
                                xk_t[c][:, qc * QCW:(qc + 1) * QCW],
                                start=(c == 0), stop=(c == NDC - 1),
                            )
                    for t in range(2):
                        nc.vector.tensor_scalar_add(
                            qt_sb[t][:, qc * QCW:(qc + 1) * QCW], pq[t][:],
                            bq_sb[:, t:t + 1],
                        )
                        nc.vector.tensor_scalar_add(
                            kt_sb[t][:, qc * QCW:(qc + 1) * QCW], pk[t][:],
                            bk_sb[:, t:t + 1],
                        )

            # phase C: v in natural [L, HD] layout, +bias, ones col pre-set
            with tc.tile_pool(name="pvp", bufs=2, space="PSUM") as pvp:
                for lg in range(4):  # groups of 4 k-blocks
                    pv = [pvp.tile([128, HD], f32, tag=f"pv{j}", name=f"pv{j}")
                          for j in range(4)]
                    for c in range(NDC):
                        for j in range(4):
                            nc.tensor.matmul(
                                pv[j][:],
                                xv_t[c][:, (lg * 4 + j) * KBW:(lg * 4 + j + 1) * KBW],
                                wv_sb[:, c * HD:(c + 1) * HD],
                                start=(c == 0), stop=(c == NDC - 1),
                            )
                    for j in range(4):
                        lb = lg * 4 + j
                        ov = vaug_sb[lb][:].rearrange("p (h c) -> p h c", c=66)[:, :, 0:64]
                        iv = pv[j][:].rearrange("p (h c) -> p h c", c=64)
                        bv_v = bvt_sb[:].rearrange("p (h c) -> p h c", c=64)
                        nc.vector.tensor_add(ov, iv, bv_v)

        if stop_after == "C":
            for t in range(2):
                nc.sync.dma_start(yt[t * 128:(t + 1) * 128, :], qt_sb[t][:])
                nc.sync.dma_start(yt[256 + t * 128:256 + (t + 1) * 128, :], kt_sb[t][:])
            for lb in range(NKB):
                nc.sync.dma_start(
                    yt[512 + (lb % 4) * 128: 512 + (lb % 4 + 1) * 128,
                       (lb // 4) * 512:(lb // 4) * 512 + HPC * 66],
                    vaug_sb[lb][:],
                )
        # ---- phase D: attention ----
        run_d = stop_after not in ("C", "noD")
        with tc.tile_pool(name="psp", bufs=2, space="PSUM") as psp, \
             tc.tile_pool(name="pop", bufs=1, space="PSUM") as pop, \
             tc.tile_pool(name="ptp", bufs=6) as ptp, \
             tc.tile_pool(name="rcp", bufs=4) as rcp:
            # 4 accumulator banks: 2 ping-pong sets of 2 (one per qc in pair)
            pouts = [pop.tile([66, QCW], f32, tag=f"pout{i}", name=f"pout{i}")
                     for i in range(4)]
            pending_recip = []  # (S, t, po, qc) reciprocal right after pair end
            pending_norm = []   # rest of normalization, flushed mid-next-pair

            recip_ops = []  # micro-op queue: one DVE op drained per kb step

            def _flush_recip():
                # split each 3.3us reciprocal into two [1,256] halves emitted
                # as separate micro-ops: in-loop DVE band-muls then queue
                # behind at most ~1.7us of reciprocal work instead of ~6.6us
                while pending_recip:
                    Sj, t_, po_, qc_ = pending_recip.pop(0)
                    # rows 0..63 of Sj are outT, row 64 is the softmax sum
                    rec32 = rcp.tile([1, QCW], f32, tag="rec32", name="rec32")
                    h = QCW // 2
                    recip_ops.append(lambda Sj=Sj, rec32=rec32:
                        nc.vector.reciprocal(rec32[0:1, 0:h], Sj[64:65, 0:h]))
                    recip_ops.append(lambda Sj=Sj, rec32=rec32:
                        nc.vector.reciprocal(rec32[0:1, h:QCW], Sj[64:65, h:QCW]))

                    def _cast(Sj=Sj, rec32=rec32, t_=t_, po_=po_, qc_=qc_):
                        rec = rcp.tile([1, QCW], bf16, tag="rec", name="rec")
                        nc.vector.tensor_copy(rec[:], rec32[:])
                        pending_norm.append((Sj, rec, t_, po_, qc_))
                    recip_ops.append(_cast)

            def _drain_recip(limit=1):
                while recip_ops and limit > 0:
                    limit -= 1
                    recip_ops.pop(0)()

            def _flush_norm(limit=99):
                while pending_norm and limit > 0:
                    limit -= 1
                    Sj, rec, t_, po_, qc_ = pending_norm.pop(0)
                    pbt = psp.tile([128, 2 * QCW], f32, tag="ps", name="pbt")
                    pb = pbt[0:64, 0:QCW]
                    nc.tensor.matmul(pb, on1_sb[:], rec[:], start=True, stop=True)
                    bc = rcp.tile([64, QCW], f32, tag="bc", name="bc")
                    nc.vector.tensor_copy(bc[:], pb)
                    nc.vector.tensor_mul(
                        otp_sb[t_][po_:po_ + 64, qc_ * QCW:(qc_ + 1) * QCW],
                        Sj[0:64, :], bc[:],
                    )

            # head pairs: even head hA=2t in kt/qt rows 0:64, odd hB=2t+1 in
            # rows 64:128. Each score matmul splits into two 64x64-quadrant
            # halves on disjoint (row,col) PE tiles, which execute
            # concurrently (~1.7x measured); both heads' scores share one
            # [128,1024] psum tile (two bank-aligned halves) so a single exp
            # covers them. The whole phase is one flat software-pipelined
            # stream over (pair, kb): attn@v lags scores by 2 steps globally,
            # crossing pair boundaries, so neither PE nor ACT ever drains.
            pairs = [(t, qc) for t in range(2) for qc in range(NQC)]
            Ssets = {p: pouts[(p % 2) * 2:(p % 2) * 2 + 2] for p in range(len(pairs))}
            pts = {}

            def _scores(p, kb):
                t, qc = pairs[p]
                ps = psp.tile([128, 2 * QCW], f32, tag="ps", name="ps")
                k0 = kb * KBW
                q_sl = slice(qc * QCW, (qc + 1) * QCW)
                # quadrant pairs: [A(0,0) | B(64,64)], [A(0,64) | B(64,0)]
                for co in (0, 64):
                    nc.tensor.matmul(
                        ps[co:co + 64, 0:QCW],
                        kt_sb[t][0:64, k0 + co: k0 + co + 64],
                        qt_sb[t][0:64, q_sl],
                        start=True, stop=True,
                    )
                    nc.tensor.matmul(
                        ps[64 - co:128 - co, QCW:2 * QCW],
                        kt_sb[t][64:128, k0 + 64 - co: k0 + 128 - co],
                        qt_sb[t][64:128, q_sl],
                        start=True, stop=True,
                    )
                pt = ptp.tile([128, 2 * QCW], bf16, tag="pt", name="pt")
                pts[(p, kb)] = pt
                nc.scalar.activation(pt[:], ps[:], AF.Exp, scale=SCALE)
                delta = k0 - qc * QCW
                if delta in slots:
                    si, c0, c1 = slots[delta]
                    # one half per engine so the band multiply adds only half
                    # its latency to the exp -> attn@v chain
                    for j, eng in ((0, nc.vector), (1, nc.gpsimd)):
                        eng.tensor_mul(
                            pt[:, j * QCW + c0: j * QCW + c1],
                            pt[:, j * QCW + c0: j * QCW + c1],
                            em_sb[:, si * 512 + c0: si * 512 + c1],
                        )

            def _attnv(p, kb):
                t, qc = pairs[p]
                for j, hh in ((0, 2 * t), (1, 2 * t + 1)):
                    nc.tensor.matmul(
                        Ssets[p][j][:],
                        vaug_sb[kb][:, hh * 66:(hh + 1) * 66],
                        pts[(p, kb)][:, j * QCW:(j + 1) * QCW],
                        start=(kb == 0), stop=(kb == NKB - 1),
                    )
                del pts[(p, kb)]
                if kb == NKB - 1:
                    S, (t_, qc_) = Ssets[p], pairs[p]
                    pending_recip.append((S[0], t_, 0, qc_))
                    pending_recip.append((S[1], t_, 64, qc_))
                    _flush_recip()

            LAG = 3
            total = (len(pairs) * NKB) if run_d else 0
            for g in range(total):
                p, kb = divmod(g, NKB)
                _scores(p, kb)
                _drain_recip(limit=1)
                # previous pair's normalization, deep inside this pair's kb
                # loop: its PE bcast matmul enqueues with its rec long since
                # ready, so no head-of-line stall
                if kb in (6, 10):
                    _flush_norm(limit=1)
                if g >= LAG:
                    _attnv(*divmod(g - LAG, NKB))
            for g in range(max(total - LAG, 0), total):
                _attnv(*divmod(g, NKB))
            _drain_recip(limit=99)
            _flush_norm()

        if stop_after == "D":
            for t in range(2):
                nc.sync.dma_start(yt[t * 128:(t + 1) * 128, :], otp_sb[t][:].bitcast(bf16))
        run_e = stop_after in (None, "noD")
        # ---- phase E: output projection yT partial (heads paired, K=128) ----
        with tc.tile_pool(name="pyp", bufs=2, space="PSUM") as pyp, \
             tc.tile_pool(name="ysp", bufs=3) as ysp:
            for db in range(NDC if run_e else 0):
                py = pyp.tile([128, NQC * QCW], f32, tag="py", name="py")  # 4 banks
                for t in range(2):
                    for qc in range(NQC):
                        nc.tensor.matmul(
                            py[:, qc * QCW:(qc + 1) * QCW],
                            wo_sb[:, t * D + db * DCW: t * D + (db + 1) * DCW],
                            otp_sb[t][:, qc * QCW:(qc + 1) * QCW],
                            start=(t == 0), stop=(t == 1),
                        )
                y_sb = ysp.tile([128, NQC * QCW], bf16, tag="y", name="y")
                # split the psum drain across both free engines
                nc.vector.tensor_copy(y_sb[:, 0:2 * QCW], py[:, 0:2 * QCW])
                nc.scalar.copy(y_sb[:, 2 * QCW:4 * QCW], py[:, 2 * QCW:4 * QCW])
                nc.sync.dma_start(yt[db * DCW:(db + 1) * DCW, :], y_sb[:])

    nc.compile()
    return nc


def _pack_ndc(w_g):
    """[HD, D] row-slice of a Linear weight -> [128, NDC*HD] SBUF image with
    w[p, c*HD+n] = w_g[n, c*128+p] (lhsT chunks along the free dim)."""
    return np.ascontiguousarray(
        w_g.reshape(HD, NDC, 128).transpose(2, 1, 0).reshape(128, NDC * HD)
    )


def _prep_inmaps(query, key, value, Wq, bq, Wk, bk, Wv, bv, Wo, masksize):
    bfl = ml_dtypes.bfloat16
    half = int(masksize) // 2
    slots = _band_slots(half)
    ns = max(len(slots), 1)
    em = np.ones((128, ns * 512), np.float32)
    e1 = np.float32(np.exp(np.float32(1.0)))
    p = np.arange(128)[:, None]
    f = np.arange(512)[None, :]
    for d, (si, _, _) in slots.items():
        em[:, si * 512:(si + 1) * 512] = np.where(
            np.abs(d + p - f) <= half, e1, np.float32(1.0)
        )
    em = em.astype(bfl)

    xqT = [np.ascontiguousarray(query[b].T).astype(bfl) for b in range(B)]
    xkT = [np.ascontiguousarray(key[b].T).astype(bfl) for b in range(B)]
    xvT = [np.ascontiguousarray(value[b].T).astype(bfl) for b in range(B)]
    wqP = [_pack_ndc(Wq[g * HD:(g + 1) * HD, :]).astype(bfl) for g in range(4)]
    wkP = [_pack_ndc(Wk[g * HD:(g + 1) * HD, :]).astype(bfl) for g in range(4)]
    wvP = [_pack_ndc(Wv[g * HD:(g + 1) * HD, :]).astype(bfl) for g in range(4)]
    # wo[p, t*D+n] = Wo[n, g*HD + t*128 + p]  (head-pair lhsT blocks, K=128)
    woP = [
        np.ascontiguousarray(
            Wo[:, g * HD:(g + 1) * HD].reshape(D, 2, 128).transpose(2, 1, 0).reshape(128, 2 * D)
        ).astype(bfl)
        for g in range(4)
    ]
    bqP = [np.ascontiguousarray(bq[g * HD:(g + 1) * HD].reshape(2, 128).T) for g in range(4)]
    bkP = [np.ascontiguousarray(bk[g * HD:(g + 1) * HD].reshape(2, 128).T) for g in range(4)]
    bvP = [
        np.ascontiguousarray(np.tile(bv[g * HD:(g + 1) * HD], (128, 1)))
        for g in range(4)
    ]

    in_maps = []
    for c in range(8):
        b, g = c // 4, c % 4
        in_maps.append({
            "xq": xqT[b], "xk": xkT[b], "xv": xvT[b],
            "wq": wqP[g], "wk": wkP[g], "wv": wvP[g], "wo": woP[g],
            "bq": bqP[g], "bk": bkP[g], "bvt": bvP[g], "em": em,
            "on1": np.ones((1, 64), bfl),
        })
    return in_maps


def kernel(query, key, value, Wq, bq, Wk, bk, Wv, bv, Wo, bo, masksize):
    query = np.asarray(query, dtype=np.float32)
    key = np.asarray(key, dtype=np.float32)
    value = np.asarray(value, dtype=np.float32)
    Wq, bq = np.asarray(Wq, np.float32), np.asarray(bq, np.float32)
    Wk, bk = np.asarray(Wk, np.float32), np.asarray(bk, np.float32)
    Wv, bv = np.asarray(Wv, np.float32), np.asarray(bv, np.float32)
    Wo, bo = np.asarray(Wo, np.float32), np.asarray(bo, np.float32)
    ms = int(np.asarray(masksize))

    if ms not in _CACHE:
        _CACHE[ms] = _build(ms)
    nc = _CACHE[ms]

    in_maps = _prep_inmaps(query, key, value, Wq, bq, Wk, bk, Wv, bv, Wo, ms)
    res = None
    for attempt in range(3):
        try:
            res = run_bass_kernel_spmd(nc, in_maps, list(range(8)))
            break
        except Exception:
            # first execution after a fresh compile occasionally hits a
            # transient NRT_EXEC_UNIT_UNRECOVERABLE; a retry succeeds
            if attempt == 2:
                raise
            import time
            time.sleep(10)

    out = np.empty((B, L, D), np.float32)
    for b in range(B):
        acc = res.results[4 * b]["yt"].astype(np.float32)
        for g in range(1, 4):
            acc = acc + res.results[4 * b + g]["yt"].astype(np.float32)
        out[b] = acc.T + bo
    return out


# revision 43
# speedup vs baseline: 1.0132x; 1.0132x over previous
"""Banded multi-head attention on 8 Trainium2 NeuronCores.

Problem: B=2, L=2048, D=1024, H=16 heads, d_k=64. The band mask is a 0/1
FLOAT tensor *added* to the scores (not -inf masked), so the softmax is
dense; exp(s + band) = exp(s) * e^band is handled by multiplying constant
e-or-1 parallelogram tiles over the band region.

Sharding: (batch x 4-head-groups) across the 8 cores. Host pre-transposes
activations/weights so every device matmul has its contraction dim on
partitions; the per-core partial output projections are summed on the host
(gather/unshard) together with the output bias.

All device matmuls run on bf16 operands with f32 PSUM accumulation; the
tolerance (2e-2) leaves ample margin and bf16 is the PE's 1-col/cycle
fast path (fp32r lowers to the slow HIGH-precision mode on this stack).
"""

import sys

sys.path.insert(0, "/opt/trn_rl_repo")

import numpy as np
import ml_dtypes
from contextlib import ExitStack

import concourse.bass as bass
import concourse.tile as tile
from concourse import bacc, mybir
from concourse.bass_utils import run_bass_kernel_spmd

dt = mybir.dt
AF = mybir.ActivationFunctionType

B, L, D, H, DK = 2, 2048, 1024, 16, 64
HPC = 4            # heads per core
HD = HPC * DK      # 256: head dims per core
NQC, QCW = 4, 512  # q chunks
NKB, KBW = 16, 128 # k blocks
NDC, DCW = 8, 128  # D chunks
SCALE = 1.0 / 8.0  # 1/sqrt(d_k)

_CACHE = {}


def _band_slots(half):
    """delta -> (slot, c0, c1) for 128x512 tiles at k-offset kb*128, q-offset
    qc*512, delta = kb*128 - qc*512. Band cols: f in [delta-half, delta+127+half]."""
    slots = {}
    d = -((half + 127) // 128) * 128
    while d <= half + 511:
        c0, c1 = max(0, d - half), min(512, d + 128 + half)
        if c0 < c1:
            slots[d] = (len(slots), c0, c1)
        d += 128
    return slots


def _build(masksize, stop_after=None):
    half = int(masksize) // 2
    slots = _band_slots(half)
    ns = max(len(slots), 1)

    nc = bacc.Bacc("TRN2", target_bir_lowering=False, debug=False)

    f32, bf16 = dt.float32, dt.bfloat16
    xq = nc.dram_tensor("xq", [D, L], bf16, kind="ExternalInput").ap()
    xk = nc.dram_tensor("xk", [D, L], bf16, kind="ExternalInput").ap()
    xv = nc.dram_tensor("xv", [D, L], bf16, kind="ExternalInput").ap()
    # weights pre-packed on host into SBUF layouts (see _prep_inmaps)
    wq = nc.dram_tensor("wq", [128, NDC * HD], bf16, kind="ExternalInput").ap()
    wk = nc.dram_tensor("wk", [128, NDC * HD], bf16, kind="ExternalInput").ap()
    wv = nc.dram_tensor("wv", [128, NDC * HD], bf16, kind="ExternalInput").ap()
    wo = nc.dram_tensor("wo", [128, 2 * D], bf16, kind="ExternalInput").ap()
    bq = nc.dram_tensor("bq", [128, 2], f32, kind="ExternalInput").ap()
    bk = nc.dram_tensor("bk", [128, 2], f32, kind="ExternalInput").ap()
    bvt = nc.dram_tensor("bvt", [128, HD], f32, kind="ExternalInput").ap()
    em = nc.dram_tensor("em", [128, ns * 512], bf16, kind="ExternalInput").ap()
    on1 = nc.dram_tensor("on1", [1, 64], bf16, kind="ExternalInput").ap()
    yt = nc.dram_tensor("yt", [D, L], bf16, kind="ExternalOutput").ap()

    with tile.TileContext(nc) as tc, ExitStack() as ctx:
        ctx.enter_context(
            nc.allow_low_precision(reason="bf16 matmul operands are intentional")
        )
        # ---- persistent SBUF ----
        wts = ctx.enter_context(tc.tile_pool(name="wts", bufs=1))
        big = ctx.enter_context(tc.tile_pool(name="big", bufs=1))

        wq_sb = wts.tile([128, NDC * HD], bf16, tag="wq", name="wq")
        wk_sb = wts.tile([128, NDC * HD], bf16, tag="wk", name="wk")
        wv_sb = wts.tile([128, NDC * HD], bf16, tag="wv", name="wv")
        wo_sb = wts.tile([128, 2 * D], bf16, tag="wo", name="wo")
        bq_sb = wts.tile([128, 2], f32, tag="bq", name="bq")
        bk_sb = wts.tile([128, 2], f32, tag="bk", name="bk")
        bvt_sb = wts.tile([128, HD], f32, tag="bvt", name="bvt")
        em_sb = wts.tile([128, ns * 512], bf16, tag="em", name="em")
        on1_sb = wts.tile([1, 64], bf16, tag="on1", name="on1")
        # DMA issue order is managed below: phase-B weights first, then the
        # xq/xk stream, then everything else in phase order

        # projection outputs (resident): q/k in T-layout, out paired by heads
        qt_sb = [big.tile([128, L], bf16, tag=f"qt{t}", name=f"qt{t}") for t in range(2)]
        kt_sb = [big.tile([128, L], bf16, tag=f"kt{t}", name=f"kt{t}") for t in range(2)]
        otp_sb = [big.tile([128, L], bf16, tag=f"ot{t}", name=f"ot{t}") for t in range(2)]
        # v (natural layout) + ones col per head: [128, HPC*66] per k-block
        vaug_sb = [big.tile([128, HPC * 66], bf16, tag=f"vaug{lb}", name=f"vaug{lb}") for lb in range(NKB)]
        for lb in range(NKB):
            nc.gpsimd.memset(vaug_sb[lb][:], 1.0)

        # ---- phases B/C: projections with resident x chunks ----
        with tc.tile_pool(name="xs", bufs=1) as xs:
            for t_sb, t_in in ((wq_sb, wq), (wk_sb, wk)):
                nc.sync.dma_start(t_sb[:], t_in[:])
            # xq/xk chunks first (phase B needs them all); xv streams in
            # during phase B compute and is only needed for phase C
            xq_t, xk_t, xv_t = [], [], []
            for c in range(NDC):
                for lst, nm, src in ((xq_t, "xq", xq), (xk_t, "xk", xk)):
                    t_ = xs.tile([128, L], bf16, tag=f"{nm}{c}", name=f"{nm}{c}")
                    nc.sync.dma_start(t_[:], src[c * DCW:(c + 1) * DCW, :])
                    lst.append(t_)
            for t_sb, t_in in ((bq_sb, bq), (bk_sb, bk), (wv_sb, wv),
                               (bvt_sb, bvt), (em_sb, em)):
                nc.sync.dma_start(t_sb[:], t_in[:])
            for c in range(NDC):
                t_ = xs.tile([128, L], bf16, tag=f"xv{c}", name=f"xv{c}")
                nc.sync.dma_start(t_[:], xv[c * DCW:(c + 1) * DCW, :])
                xv_t.append(t_)
            for t_sb, t_in in ((wo_sb, wo), (on1_sb, on1)):
                nc.sync.dma_start(t_sb[:], t_in[:])

            # phase B: q/k projections (T-layout)
            with tc.tile_pool(name="pbk", bufs=2, space="PSUM") as pbk:
                for qc in range(NQC):
                    pq = [pbk.tile([128, QCW], f32, tag=f"pq{t}", name=f"pq{t}") for t in range(2)]
                    pk = [pbk.tile([128, QCW], f32, tag=f"pk{t}", name=f"pk{t}") for t in range(2)]
                    for c in range(NDC):
                        for t in range(2):
                            nc.tensor.matmul(
                                pq[t][:], wq_sb[:, c * HD + t * 128: c * HD + (t + 1) * 128],
                                xq_t[c][:, qc * QCW:(qc + 1) * QCW],
                                start=(c == 0), stop=(c == NDC - 1),
                            )
                            nc.tensor.matmul(
                                pk[t][:], wk_sb[:, c * HD + t * 128: c * HD + (t + 1) * 128],
                                xk_t[c][:, qc * QCW:(qc + 1) * QCW],
                                start=(c == 0), stop=(c == NDC - 1),
                            )
                    for t in range(2):
                        nc.vector.tensor_scalar_add(
                            qt_sb[t][:, qc * QCW:(qc + 1) * QCW], pq[t][:],
                            bq_sb[:, t:t + 1],
                        )
                        nc.vector.tensor_scalar_add(
                            kt_sb[t][:, qc * QCW:(qc + 1) * QCW], pk[t][:],
                            bk_sb[:, t:t + 1],
                        )

            # phase C: v in natural [L, HD] layout, +bias, ones col pre-set
            with tc.tile_pool(name="pvp", bufs=2, space="PSUM") as pvp:
                for lg in range(4):  # groups of 4 k-blocks
                    pv = [pvp.tile([128, HD], f32, tag=f"pv{j}", name=f"pv{j}")
                          for j in range(4)]
                    for c in range(NDC):
                        for j in range(4):
                            nc.tensor.matmul(
                                pv[j][:],
                                xv_t[c][:, (lg * 4 + j) * KBW:(lg * 4 + j + 1) * KBW],
                                wv_sb[:, c * HD:(c + 1) * HD],
                                start=(c == 0), stop=(c == NDC - 1),
                            )
                    for j in range(4):
                        lb = lg * 4 + j
                        ov = vaug_sb[lb][:].rearrange("p (h c) -> p h c", c=66)[:, :, 0:64]
                        iv = pv[j][:].rearrange("p (h c) -> p h c", c=64)
                        bv_v = bvt_sb[:].rearrange("p (h c) -> p h c", c=64)
                        nc.vector.tensor_add(ov, iv, bv_v)

        if stop_after == "C":
            for t in range(2):
                nc.sync.dma_start(yt[t * 128:(t + 1) * 128, :], qt_sb[t][:])
                nc.sync.dma_start(yt[256 + t * 128:256 + (t + 1) * 128, :], kt_sb[t][:])
            for lb in range(NKB):
                nc.sync.dma_start(
                    yt[512 + (lb % 4) * 128: 512 + (lb % 4 + 1) * 128,
                       (lb // 4) * 512:(lb // 4) * 512 + HPC * 66],
                    vaug_sb[lb][:],
                )
        # ---- phase D: attention ----
        run_d = stop_after not in ("C", "noD")
        with tc.tile_pool(name="psp", bufs=2, space="PSUM") as psp, \
             tc.tile_pool(name="pop", bufs=1, space="PSUM") as pop, \
             tc.tile_pool(name="ptp", bufs=6) as ptp, \
             tc.tile_pool(name="rcp", bufs=4) as rcp:
            # 4 accumulator banks: 2 ping-pong sets of 2 (one per qc in pair)
            pouts = [pop.tile([66, QCW], f32, tag=f"pout{i}", name=f"pout{i}")
                     for i in range(4)]
            pending_recip = []  # (S, t, po, qc) reciprocal right after pair end
            pending_norm = []   # rest of normalization, flushed mid-next-pair

            recip_ops = []  # micro-op queue: one DVE op drained per kb step

            def _flush_recip():
                # split each 3.3us reciprocal into two [1,256] halves emitted
                # as separate micro-ops: in-loop DVE band-muls then queue
                # behind at most ~1.7us of reciprocal work instead of ~6.6us
                while pending_recip:
                    Sj, t_, po_, qc_ = pending_recip.pop(0)
                    # rows 0..63 of Sj are outT, row 64 is the softmax sum
                    rec32 = rcp.tile([1, QCW], f32, tag="rec32", name="rec32")
                    h = QCW // 2
                    recip_ops.append(lambda Sj=Sj, rec32=rec32:
                        nc.vector.reciprocal(rec32[0:1, 0:h], Sj[64:65, 0:h]))
                    recip_ops.append(lambda Sj=Sj, rec32=rec32:
                        nc.vector.reciprocal(rec32[0:1, h:QCW], Sj[64:65, h:QCW]))

                    def _cast(Sj=Sj, rec32=rec32, t_=t_, po_=po_, qc_=qc_):
                        rec = rcp.tile([1, QCW], bf16, tag="rec", name="rec")
                        nc.vector.tensor_copy(rec[:], rec32[:])
                        pending_norm.append((Sj, rec, t_, po_, qc_))
                    recip_ops.append(_cast)

            def _drain_recip(limit=1):
                while recip_ops and limit > 0:
                    limit -= 1
                    recip_ops.pop(0)()

            def _flush_norm(limit=99):
                while pending_norm and limit > 0:
                    limit -= 1
                    Sj, rec, t_, po_, qc_ = pending_norm.pop(0)
                    pbt = psp.tile([128, 2 * QCW], f32, tag="ps", name="pbt")
                    pb = pbt[0:64, 0:QCW]
                    nc.tensor.matmul(pb, on1_sb[:], rec[:], start=True, stop=True)
                    bc = rcp.tile([64, QCW], f32, tag="bc", name="bc")
                    nc.vector.tensor_copy(bc[:], pb)
                    nc.vector.tensor_mul(
                        otp_sb[t_][po_:po_ + 64, qc_ * QCW:(qc_ + 1) * QCW],
                        Sj[0:64, :], bc[:],
                    )

            # head pairs: even head hA=2t in kt/qt rows 0:64, odd hB=2t+1 in
            # rows 64:128. Each score matmul splits into two 64x64-quadrant
            # halves on disjoint (row,col) PE tiles, which execute
            # concurrently (~1.7x measured); both heads' scores share one
            # [128,1024] psum tile (two bank-aligned halves) so a single exp
            # covers them. The whole phase is one flat software-pipelined
            # stream over (pair, kb): attn@v lags scores by 2 steps globally,
            # crossing pair boundaries, so neither PE nor ACT ever drains.
            pairs = [(t, qc) for t in range(2) for qc in range(NQC)]
            Ssets = {p: pouts[(p % 2) * 2:(p % 2) * 2 + 2] for p in range(len(pairs))}
            pts = {}

            def _scores(p, kb):
                t, qc = pairs[p]
                ps = psp.tile([128, 2 * QCW], f32, tag="ps", name="ps")
                k0 = kb * KBW
                q_sl = slice(qc * QCW, (qc + 1) * QCW)
                # quadrant pairs: [A(0,0) | B(64,64)], [A(0,64) | B(64,0)]
                for co in (0, 64):
                    nc.tensor.matmul(
                        ps[co:co + 64, 0:QCW],
                        kt_sb[t][0:64, k0 + co: k0 + co + 64],
                        qt_sb[t][0:64, q_sl],
                        start=True, stop=True,
                    )
                    nc.tensor.matmul(
                        ps[64 - co:128 - co, QCW:2 * QCW],
                        kt_sb[t][64:128, k0 + 64 - co: k0 + 128 - co],
                        qt_sb[t][64:128, q_sl],
                        start=True, stop=True,
                    )
                pt = ptp.tile([128, 2 * QCW], bf16, tag="pt", name="pt")
                pts[(p, kb)] = pt
                nc.scalar.activation(pt[:], ps[:], AF.Exp, scale=SCALE)
                delta = k0 - qc * QCW
                if delta in slots:
                    si, c0, c1 = slots[delta]
                    # one half per engine so the band multiply adds only half
                    # its latency to the exp -> attn@v chain
                    for j, eng in ((0, nc.vector), (1, nc.gpsimd)):
                        eng.tensor_mul(
                            pt[:, j * QCW + c0: j * QCW + c1],
                            pt[:, j * QCW + c0: j * QCW + c1],
                            em_sb[:, si * 512 + c0: si * 512 + c1],
                        )

            def _attnv(p, kb):
                t, qc = pairs[p]
                for j, hh in ((0, 2 * t), (1, 2 * t + 1)):
                    nc.tensor.matmul(
                        Ssets[p][j][:],
                        vaug_sb[kb][:, hh * 66:(hh + 1) * 66],
                        pts[(p, kb)][:, j * QCW:(j + 1) * QCW],
                        start=(kb == 0), stop=(kb == NKB - 1),
                    )
                del pts[(p, kb)]
                if kb == NKB - 1:
                    S, (t_, qc_) = Ssets[p], pairs[p]
                    pending_recip.append((S[0], t_, 0, qc_))
                    pending_recip.append((S[1], t_, 64, qc_))
                    _flush_recip()

            LAG = 2
            total = (len(pairs) * NKB) if run_d else 0
            for g in range(total):
                p, kb = divmod(g, NKB)
                _scores(p, kb)
                _drain_recip(limit=1)
                # previous pair's normalization, deep inside this pair's kb
                # loop: its PE bcast matmul enqueues with its rec long since
                # ready, so no head-of-line stall
                if kb in (6, 10):
                    _flush_norm(limit=1)
                if g >= LAG:
                    _attnv(*divmod(g - LAG, NKB))
            for g in range(max(total - LAG, 0), total):
                _attnv(*divmod(g, NKB))
            _drain_recip(limit=99)
            _flush_norm()

        if stop_after == "D":
            for t in range(2):
                nc.sync.dma_start(yt[t * 128:(t + 1) * 128, :], otp_sb[t][:].bitcast(bf16))
        run_e = stop_after in (None, "noD")
        # ---- phase E: output projection yT partial (heads paired, K=128) ----
        with tc.tile_pool(name="pyp", bufs=2, space="PSUM") as pyp, \
             tc.tile_pool(name="ysp", bufs=3) as ysp:
            for db in range(NDC if run_e else 0):
                py = pyp.tile([128, NQC * QCW], f32, tag="py", name="py")  # 4 banks
                for t in range(2):
                    for qc in range(NQC):
                        nc.tensor.matmul(
                            py[:, qc * QCW:(qc + 1) * QCW],
                            wo_sb[:, t * D + db * DCW: t * D + (db + 1) * DCW],
                            otp_sb[t][:, qc * QCW:(qc + 1) * QCW],
                            start=(t == 0), stop=(t == 1),
                        )
                y_sb = ysp.tile([128, NQC * QCW], bf16, tag="y", name="y")
                # split the psum drain across both free engines
                nc.vector.tensor_copy(y_sb[:, 0:2 * QCW], py[:, 0:2 * QCW])
                nc.scalar.copy(y_sb[:, 2 * QCW:4 * QCW], py[:, 2 * QCW:4 * QCW])
                nc.sync.dma_start(yt[db * DCW:(db + 1) * DCW, :], y_sb[:])

    nc.compile()
    return nc


def _pack_ndc(w_g):
    """[HD, D] row-slice of a Linear weight -> [128, NDC*HD] SBUF image with
    w[p, c*HD+n] = w_g[n, c*128+p] (lhsT chunks along the free dim)."""
    return np.ascontiguousarray(
        w_g.reshape(HD, NDC, 128).transpose(2, 1, 0).reshape(128, NDC * HD)
    )


def _prep_inmaps(query, key, value, Wq, bq, Wk, bk, Wv, bv, Wo, masksize):
    bfl = ml_dtypes.bfloat16
    half = int(masksize) // 2
    slots = _band_slots(half)
    ns = max(len(slots), 1)
    em = np.ones((128, ns * 512), np.float32)
    e1 = np.float32(np.exp(np.float32(1.0)))
    p = np.arange(128)[:, None]
    f = np.arange(512)[None, :]
    for d, (si, _, _) in slots.items():
        em[:, si * 512:(si + 1) * 512] = np.where(
            np.abs(d + p - f) <= half, e1, np.float32(1.0)
        )
    em = em.astype(bfl)

    xqT = [np.ascontiguousarray(query[b].T).astype(bfl) for b in range(B)]
    xkT = [np.ascontiguousarray(key[b].T).astype(bfl) for b in range(B)]
    xvT = [np.ascontiguousarray(value[b].T).astype(bfl) for b in range(B)]
    wqP = [_pack_ndc(Wq[g * HD:(g + 1) * HD, :]).astype(bfl) for g in range(4)]
    wkP = [_pack_ndc(Wk[g * HD:(g + 1) * HD, :]).astype(bfl) for g in range(4)]
    wvP = [_pack_ndc(Wv[g * HD:(g + 1) * HD, :]).astype(bfl) for g in range(4)]
    # wo[p, t*D+n] = Wo[n, g*HD + t*128 + p]  (head-pair lhsT blocks, K=128)
    woP = [
        np.ascontiguousarray(
            Wo[:, g * HD:(g + 1) * HD].reshape(D, 2, 128).transpose(2, 1, 0).reshape(128, 2 * D)
        ).astype(bfl)
        for g in range(4)
    ]
    bqP = [np.ascontiguousarray(bq[g * HD:(g + 1) * HD].reshape(2, 128).T) for g in range(4)]
    bkP = [np.ascontiguousarray(bk[g * HD:(g + 1) * HD].reshape(2, 128).T) for g in range(4)]
    bvP = [
        np.ascontiguousarray(np.tile(bv[g * HD:(g + 1) * HD], (128, 1)))
        for g in range(4)
    ]

    in_maps = []
    for c in range(8):
        b, g = c // 4, c % 4
        in_maps.append({
            "xq": xqT[b], "xk": xkT[b], "xv": xvT[b],
            "wq": wqP[g], "wk": wkP[g], "wv": wvP[g], "wo": woP[g],
            "bq": bqP[g], "bk": bkP[g], "bvt": bvP[g], "em": em,
            "on1": np.ones((1, 64), bfl),
        })
    return in_maps


def kernel(query, key, value, Wq, bq, Wk, bk, Wv, bv, Wo, bo, masksize):
    query = np.asarray(query, dtype=np.float32)
    key = np.asarray(key, dtype=np.float32)
    value = np.asarray(value, dtype=np.float32)
    Wq, bq = np.asarray(Wq, np.float32), np.asarray(bq, np.float32)
    Wk, bk = np.asarray(Wk, np.float32), np.asarray(bk, np.float32)
    Wv, bv = np.asarray(Wv, np.float32), np.asarray(bv, np.float32)
    Wo, bo = np.asarray(Wo, np.float32), np.asarray(bo, np.float32)
    ms = int(np.asarray(masksize))

    if ms not in _CACHE:
        _CACHE[ms] = _build(ms)
    nc = _CACHE[ms]

    in_maps = _prep_inmaps(query, key, value, Wq, bq, Wk, bk, Wv, bv, Wo, ms)
    res = None
    for attempt in range(3):
        try:
            res = run_bass_kernel_spmd(nc, in_maps, list(range(8)))
            break
        except Exception:
            # first execution after a fresh compile occasionally hits a
            # transient NRT_EXEC_UNIT_UNRECOVERABLE; a retry succeeds
            if attempt == 2:
                raise
            import time
            time.sleep(10)

    out = np.empty((B, L, D), np.float32)
    for b in range(B):
        acc = res.results[4 * b]["yt"].astype(np.float32)
        for g in range(1, 4):
            acc = acc + res.results[4 * b + g]["yt"].astype(np.float32)
        out[b] = acc.T + bo
    return out
